# revision 8
# baseline (speedup 1.0000x reference)
"""AssociativeMemoryStep kernel for 8 TRN2 NeuronCores.

Math: the reference is LINEAR (no softmax) anti-causal attention:
    out[b,t] = (sum_{s>t} decay^{s-t-1} (q_t.k_s) v_s) @ o_w.T * out_scale
with decay = sigmoid(decay_logit) ~= 0.9526, so contributions vanish
below f32 noise within ~256 tokens.  Each core processes an independent
2048-token slice with a 128-token right halo -- fully data-parallel.

Everything factors through the 128-dim Fourier basis space:
    xb  = basis^T x^T                      [128, T]
    S^T = xb^T G xb,  G = kco qco^T        (Gram matrix in basis space)
    rb  = (xb^T P)^T (decay_mask * S^T),  P = vco oco
    y   = rb^T @ basis^T
so the C=256 channel dim never materializes on chip.  G and P are
[128,128] input-only transforms, precomputed on host.

The TRN2 PE has a DVFS ramp: it only reaches 2.4 GHz after ~3us of
continuous execution and droops back on ~us idle gaps.  A ~3us warmup
burst covers the DMA-latency head, and keep-alive matmuls bridge the
one known input-stream stall (waiting for chunk 2); after that the
attention bands keep the PE saturated.

Schedule: input x streams in 6 chunks; per input band the kernel runs
projections (xb/gq/vo) then the attention pairs (scores S, retrieve PV,
output projection Y) whose key range that band completes, so compute
chases the input stream and output DMA overlaps the input tail.
"""

import os
import numpy as np

# ---- problem constants (hardcoded per harness spec) ----
B, T, V = 4, 4096, 1024
NB2 = 128          # 2 * n_basis
C = 256            # channels
N_CORES = 8
T_OUT = 2048       # output tokens per core
W = 128            # halo (decay**128 ~ 2e-3, below the f16 noise floor)
T_LOC = T_OUT + W  # 2176 tokens held per core
N_DIAG = 2         # key band = 2 diagonal 128-blocks (>=128-token window)
T_CHUNKS = [128, 256, 512, 512, 512, 256]   # ramp-in then steady, sum 2176
N_BLK = T_LOC // 128   # 17 vo blocks
Y_SCALE = 16.0     # output emitted as f16 at 1/16 scale (f16 range guard)
N_WARM = int(os.environ.get('KW', 26))        # ~2.8us PE warmup (DVFS ramp + DMA-latency head)
N_KA = int(os.environ.get('KK', 12))          # keep-alive matmuls bridging the chunk-2 input stall

LAST = {}


def _build_nc():
    import concourse.tile as tile
    from concourse import bacc, mybir
    from contextlib import ExitStack

    f32 = mybir.dt.float32
    f16 = mybir.dt.float16

    nc = bacc.Bacc()
    xt_d = nc.declare_dram_parameter("xtp", [128, 8 * T_LOC], f16, isOutput=False)
    basis_d = nc.declare_dram_parameter("basisp", [128, 8 * NB2], f16, isOutput=False)
    basisT_d = nc.declare_dram_parameter("basisT", [NB2, V], f16, isOutput=False)
    gp_d = nc.declare_dram_parameter("gpp", [128, 2 * 128], f16, isOutput=False)
    mask2_d = nc.declare_dram_parameter("mask2", [128, N_DIAG * 128], f16, isOutput=False)
    out_d = nc.declare_dram_parameter("out", [T_OUT, V], f16, isOutput=True)

    with ExitStack() as ctx:
        tc = ctx.enter_context(tile.TileContext(nc))
        const = ctx.enter_context(tc.tile_pool(name="const", bufs=1))
        persist = ctx.enter_context(tc.tile_pool(name="persist", bufs=1))
        xt_pool = ctx.enter_context(tc.tile_pool(name="xt", bufs=3))
        sT_pool = ctx.enter_context(tc.tile_pool(name="sT", bufs=4))
        rb_pool = ctx.enter_context(tc.tile_pool(name="rb", bufs=3))
        y_pool = ctx.enter_context(tc.tile_pool(name="y", bufs=4))
        ps = ctx.enter_context(tc.tile_pool(name="ps", bufs=2, space="PSUM"))
        pss = ctx.enter_context(tc.tile_pool(name="pss", bufs=2, space="PSUM"))
        py = ctx.enter_context(tc.tile_pool(name="py", bufs=2, space="PSUM"))

        # ---- DMA issue order == stream priority: first compute needs first ----
        def xchunk_dma(tci):
            t0 = sum(T_CHUNKS[:tci])
            tw = T_CHUNKS[tci]
            xt_t = xt_pool.tile([128, 8, tw], f16, tag="xt")
            nc.sync.dma_start(
                xt_t[:],
                xt_d[:, 8 * t0 : 8 * (t0 + tw)].rearrange("p (vt t) -> p vt t", vt=8),
            )
            return xt_t

        xt_tiles = {0: xchunk_dma(0)}
        basis_sb = const.tile([128, 8, 128], f16)
        nc.sync.dma_start(basis_sb[:], basis_d.rearrange("p (vt n) -> p vt n", vt=8))
        xt_tiles[1] = xchunk_dma(1)
        gp_sb = const.tile([128, 2, 128], f16)
        nc.sync.dma_start(gp_sb[:], gp_d.rearrange("p (ct n) -> p ct n", ct=2))
        mask4_sb = const.tile([128, 2 * N_DIAG * 128], f16)
        nc.sync.dma_start(mask4_sb[:, : N_DIAG * 128], mask2_d[:])
        nc.sync.dma_start(mask4_sb[:, N_DIAG * 128 :], mask2_d[:])
        basisT_sb = const.tile([128, V], f16)
        nc.sync.dma_start(basisT_sb[:], basisT_d[:])
        xt_tiles[2] = xchunk_dma(2)
        xt_tiles[3] = xchunk_dma(3)
        xt_tiles[4] = xchunk_dma(4)
        xt_tiles[5] = xchunk_dma(5)

        # ---- persistent activations ----
        xb_sb = persist.tile([128, T_LOC], f16)              # basis-space x^T
        gq_sb = persist.tile([128, T_OUT], f16)              # G'^T xb
        vo_sb = persist.tile([128, N_BLK, 128], f16)         # xb^T P (t-major)

        # PE warmup / keep-alive: dummy matmuls on a memset scratch tile.
        wu_sb = const.tile([128, 256], f16)
        nc.gpsimd.memset(wu_sb[:], 0.0)

        def keep_alive(n):
            for _ in range(n):
                ka_ps = ps.tile([128, 512], f32, tag="mm")
                nc.tensor.matmul(
                    ka_ps[:, 0:128], wu_sb[:, 0:128], wu_sb[:, 128:256],
                    start=True, stop=True,
                )

        def project_xb(tci):
            t0 = sum(T_CHUNKS[:tci])
            tw = T_CHUNKS[tci]
            xt_t = xt_tiles.pop(tci)
            xb_ps = ps.tile([128, tw], f32, tag="mm")
            for vt in range(8):
                nc.tensor.matmul(
                    xb_ps[:], basis_sb[:, vt, :], xt_t[:, vt, :],
                    start=(vt == 0), stop=(vt == 7),
                )
            # first two chunk evacs on DVE: ACT is busy with its one-time
            # activation-table load at kernel start.
            if tci < 2:
                nc.vector.tensor_copy(xb_sb[:, t0 : t0 + tw], xb_ps[:])
            else:
                nc.scalar.copy(xb_sb[:, t0 : t0 + tw], xb_ps[:])

        def project_gq(tci):
            t0 = sum(T_CHUNKS[:tci])
            tw = min(T_CHUNKS[tci], T_OUT - t0)
            if tw <= 0:
                return
            gq_ps = ps.tile([128, tw], f32, tag="mm")
            nc.tensor.matmul(
                gq_ps[:], gp_sb[:, 0, :], xb_sb[:, t0 : t0 + tw],
                start=True, stop=True,
            )
            if tci < 2:
                nc.vector.tensor_copy(gq_sb[:, t0 : t0 + tw], gq_ps[:])
            else:
                nc.scalar.copy(gq_sb[:, t0 : t0 + tw], gq_ps[:])

        def project_vo(b0, nb):
            # nb vo blocks ([128,128] each) batched into one PSUM bank and
            # evacuated with a single wide ACT copy.
            vo_ps = ps.tile([128, nb * 128], f32, tag="mm")
            for i in range(nb):
                a = (b0 + i) * 128
                nc.tensor.matmul(
                    vo_ps[:, i * 128 : (i + 1) * 128],
                    xb_sb[:, a : a + 128], gp_sb[:, 1, :],
                    start=(i == 0), stop=(i == nb - 1),
                )
            nc.scalar.copy(
                vo_sb[:, b0 : b0 + nb, :].rearrange("p b n -> p (b n)"), vo_ps[:]
            )

        # ---- attention, two query-chunks per stage ----
        sT_q = {}
        rb_q = {}

        def stage_s(pi):
            q0 = pi * 256
            s_ps = pss.tile([128, 4 * 128], f32, tag="s")
            first = True
            for half in range(2):
                for d in range(N_DIAG):
                    s0 = q0 + half * 128 + d * 128
                    nc.tensor.matmul(
                        s_ps[:, (half * 2 + d) * 128 : (half * 2 + d + 1) * 128],
                        xb_sb[:, s0 : s0 + 128],
                        gq_sb[:, q0 + half * 128 : q0 + (half + 1) * 128],
                        start=first, stop=(half == 1 and d == N_DIAG - 1),
                    )
                    first = False
            sT_sb = sT_pool.tile([128, 4 * 128], f16, tag="sT")
            nc.vector.tensor_mul(sT_sb[:], s_ps[:], mask4_sb[:])
            sT_q[pi] = sT_sb

        def stage_pv(pi):
            q0 = pi * 256
            sT_sb = sT_q.pop(pi)
            rb_ps4 = pss.tile([128, 512], f32, tag="s")
            rb_ps = rb_ps4[:, 0:256]
            first = True
            for half in range(2):
                for d in range(N_DIAG):
                    nc.tensor.matmul(
                        rb_ps[:, half * 128 : (half + 1) * 128],
                        vo_sb[:, q0 // 128 + half + d, :],
                        sT_sb[:, (half * 2 + d) * 128 : (half * 2 + d + 1) * 128],
                        start=first, stop=(half == 1 and d == N_DIAG - 1),
                    )
                    first = False
            rb_sb = rb_pool.tile([128, 256], f16)
            nc.scalar.copy(rb_sb[:], rb_ps[:])
            rb_q[pi] = rb_sb

        def stage_y(pi):
            q0 = pi * 256
            rb_sb = rb_q.pop(pi)
            for half in range(2):
                y_ps = py.tile([128, 1024], f32, tag="y")
                for vh in range(2):
                    nc.tensor.matmul(
                        y_ps[:, vh * 512 : (vh + 1) * 512],
                        rb_sb[:, half * 128 : (half + 1) * 128],
                        basisT_sb[:, vh * 512 : (vh + 1) * 512],
                        start=True, stop=True, skip_group_check=True,
                    )
                y_sb = y_pool.tile([128, V], f16)
                if half == 0:
                    nc.vector.tensor_copy(y_sb[:], y_ps[:])
                else:
                    nc.scalar.copy(y_sb[:], y_ps[:])
                nc.sync.dma_start(
                    out_d[q0 + half * 128 : q0 + (half + 1) * 128, :], y_sb[:]
                )

        def band(pairs):
            for pi in pairs:
                stage_s(pi)
                stage_pv(pi)
                stage_y(pi)

        # ---- emission (priority) order ----
        keep_alive(N_WARM)
        project_xb(0); project_gq(0)
        project_xb(1); project_gq(1)
        keep_alive(N_KA)          # bridge the chunk-2 stream stall
        project_xb(2); project_gq(2)
        project_vo(0, 4)
        stage_s(0)
        project_xb(3); project_gq(3)
        stage_pv(0)
        stage_s(1)
        project_vo(4, 4)
        stage_y(0)
        stage_pv(1)
        stage_s(2)
        project_xb(4); project_gq(4)
        stage_y(1)
        stage_pv(2)
        stage_s(3)
        project_vo(8, 4)
        stage_y(2)
        stage_pv(3)
        stage_s(4)
        project_xb(5); project_gq(5)
        stage_y(3)
        stage_pv(4)
        stage_s(5)
        project_vo(12, 4)
        project_vo(16, 1)
        stage_y(4)
        stage_pv(5)
        stage_s(6)
        stage_y(5)
        stage_pv(6)
        stage_s(7)
        stage_y(6)
        stage_pv(7)
        stage_y(7)

    nc.compile()
    return nc


_NC_CACHE = None


def _get_nc():
    global _NC_CACHE
    if _NC_CACHE is None:
        _NC_CACHE = _build_nc()
    return _NC_CACHE


def kernel(x, basis, q_coeffs, k_coeffs, v_coeffs, o_coeffs, decay_logit, out_scale):
    from concourse.bass_utils import run_bass_kernel_spmd

    x = np.asarray(x, dtype=np.float32)
    basis = np.ascontiguousarray(np.asarray(basis, dtype=np.float32))
    decay = float(1.0 / (1.0 + np.exp(-np.float64(np.asarray(decay_logit)))))
    oscale = float(np.asarray(out_scale))
    alpha = oscale / Y_SCALE

    p_idx = np.arange(128, dtype=np.float64)
    # full 2D key/query decay mask per diagonal block d:
    #   mask[p, qr] = alpha * decay^(d*128 + p - qr - 1),  d=0 also tri (p>qr)
    blocks = []
    for d in range(N_DIAG):
        e = d * 128.0 + p_idx[:, None] - p_idx[None, :] - 1.0
        blk = alpha * decay ** e
        if d == 0:
            blk = blk * (p_idx[:, None] > p_idx[None, :])
        blocks.append(blk)
    mask2 = np.ascontiguousarray(np.concatenate(blocks, axis=1).astype(np.float16))

    def pack_rows(a):
        # [(nt*128), m] -> [128, nt*m]  (partition-major, tile index on free)
        nt = a.shape[0] // 128
        return np.ascontiguousarray(
            a.reshape(nt, 128, a.shape[1]).transpose(1, 0, 2).reshape(128, -1)
        ).astype(np.float16)

    basisT = np.ascontiguousarray(basis.T).astype(np.float16)
    basisp = pack_rows(basis)
    # G'[n',n] = sum_c qco[c,n'] kco[c,n];  P[n,m] = sum_c vco[c,n] oco[c,m]
    qc = np.asarray(q_coeffs, dtype=np.float32)
    kc = np.asarray(k_coeffs, dtype=np.float32)
    vc = np.asarray(v_coeffs, dtype=np.float32)
    oc = np.asarray(o_coeffs, dtype=np.float32)
    gmat = (qc.T @ kc).astype(np.float16)     # [128, 128]
    pmat = (vc.T @ oc).astype(np.float16)     # [128, 128]
    gpp = np.ascontiguousarray(np.concatenate([gmat, pmat], axis=1))

    in_maps = []
    for core in range(N_CORES):
        b, h = core // 2, core % 2
        lo = h * T_OUT
        hi = min(T, lo + T_LOC)
        xs = np.zeros((T_LOC, V), dtype=np.float32)
        xs[: hi - lo] = x[b, lo:hi]
        # pack x^T into per-chunk-contiguous SBUF layout:
        # xtp[p, 8*t0 + vt*tw + t] = x[t0+t, vt*128+p] for chunk (t0, tw)
        xtt = xs.T.reshape(8, 128, T_LOC).transpose(1, 0, 2)  # [128, vt, t]
        pieces = []
        t0 = 0
        for tw in T_CHUNKS:
            pieces.append(xtt[:, :, t0 : t0 + tw].reshape(128, 8 * tw))
            t0 += tw
        xtp = np.ascontiguousarray(np.concatenate(pieces, axis=1)).astype(np.float16)
        in_maps.append(
            {
                "xtp": xtp,
                "basisp": basisp,
                "basisT": basisT,
                "gpp": gpp,
                "mask2": mask2,
            }
        )

    nc = _get_nc()
    trace = bool(int(os.environ.get("KERNEL_TRACE", "0")))
    res = run_bass_kernel_spmd(nc, in_maps, list(range(N_CORES)), trace=trace)
    LAST["exec_time_ns"] = res.exec_time_ns
    LAST["results"] = res

    out = np.empty((B, T, V), dtype=np.float32)
    for core in range(N_CORES):
        b, h = core // 2, core % 2
        out[b, h * T_OUT : (h + 1) * T_OUT] = (
            res.results[core]["out"].astype(np.float32) * Y_SCALE
        )
    return out


# revision 9
# speedup vs baseline: 1.0417x; 1.0417x over previous
"""AssociativeMemoryStep kernel for 8 TRN2 NeuronCores.

Math: the reference is LINEAR (no softmax) anti-causal attention:
    out[b,t] = (sum_{s>t} decay^{s-t-1} (q_t.k_s) v_s) @ o_w.T * out_scale
with decay = sigmoid(decay_logit) ~= 0.9526, so contributions vanish
below f32 noise within ~256 tokens.  Each core processes an independent
2048-token slice with a 128-token right halo -- fully data-parallel.

Everything factors through the 128-dim Fourier basis space:
    xb  = basis^T x^T                      [128, T]
    S^T = xb^T G xb,  G = kco qco^T        (Gram matrix in basis space)
    rb  = (xb^T P)^T (decay_mask * S^T),  P = vco oco
    y   = rb^T @ basis^T
so the C=256 channel dim never materializes on chip.  G and P are
[128,128] input-only transforms, precomputed on host.

The TRN2 PE has a DVFS ramp: it only reaches 2.4 GHz after ~3us of
continuous execution and droops back on ~us idle gaps.  A ~3us warmup
burst covers the DMA-latency head, and keep-alive matmuls bridge the
one known input-stream stall (waiting for chunk 2); after that the
attention bands keep the PE saturated.

Schedule: input x streams in 6 chunks; per input band the kernel runs
projections (xb/gq/vo) then the attention pairs (scores S, retrieve PV,
output projection Y) whose key range that band completes, so compute
chases the input stream and output DMA overlaps the input tail.
"""

import os
import numpy as np

# ---- problem constants (hardcoded per harness spec) ----
B, T, V = 4, 4096, 1024
NB2 = 128          # 2 * n_basis
C = 256            # channels
N_CORES = 8
T_OUT = 2048       # output tokens per core
W = 128            # halo (decay**128 ~ 2e-3, below the f16 noise floor)
T_LOC = T_OUT + W  # 2176 tokens held per core
N_DIAG = 2         # key band = 2 diagonal 128-blocks (>=128-token window)
T_CHUNKS = [128, 256, 512, 512, 512, 256]   # ramp-in then steady, sum 2176
N_BLK = T_LOC // 128   # 17 vo blocks
Y_SCALE = 16.0     # output emitted as f16 at 1/16 scale (f16 range guard)
N_WARM = int(os.environ.get('KW', 26))        # ~2.8us PE warmup (DVFS ramp + DMA-latency head)
N_KA = int(os.environ.get('KK', 12))          # keep-alive matmuls bridging the chunk-2 input stall

LAST = {}


def _build_nc():
    import concourse.tile as tile
    from concourse import bacc, mybir
    from contextlib import ExitStack

    f32 = mybir.dt.float32
    f16 = mybir.dt.float16

    nc = bacc.Bacc()
    xt_d = nc.declare_dram_parameter("xtp", [128, 8 * T_LOC], f16, isOutput=False)
    basis_d = nc.declare_dram_parameter("basisp", [128, 8 * NB2], f16, isOutput=False)
    basisT_d = nc.declare_dram_parameter("basisT", [NB2, V], f16, isOutput=False)
    gp_d = nc.declare_dram_parameter("gpp", [128, 2 * 128], f16, isOutput=False)
    mask2_d = nc.declare_dram_parameter("mask2", [128, N_DIAG * 128], f16, isOutput=False)
    out_d = nc.declare_dram_parameter("out", [T_OUT, V], f16, isOutput=True)

    with ExitStack() as ctx:
        tc = ctx.enter_context(tile.TileContext(nc))
        const = ctx.enter_context(tc.tile_pool(name="const", bufs=1))
        persist = ctx.enter_context(tc.tile_pool(name="persist", bufs=1))
        xt_pool = ctx.enter_context(tc.tile_pool(name="xt", bufs=3))
        sT_pool = ctx.enter_context(tc.tile_pool(name="sT", bufs=4))
        rb_pool = ctx.enter_context(tc.tile_pool(name="rb", bufs=3))
        y_pool = ctx.enter_context(tc.tile_pool(name="y", bufs=4))
        ps = ctx.enter_context(tc.tile_pool(name="ps", bufs=2, space="PSUM"))
        pss = ctx.enter_context(tc.tile_pool(name="pss", bufs=2, space="PSUM"))
        py = ctx.enter_context(tc.tile_pool(name="py", bufs=2, space="PSUM"))

        # ---- DMA issue order == stream priority: first compute needs first ----
        def xchunk_dma(tci):
            t0 = sum(T_CHUNKS[:tci])
            tw = T_CHUNKS[tci]
            xt_t = xt_pool.tile([128, 8, tw], f16, tag="xt")
            nc.sync.dma_start(
                xt_t[:],
                xt_d[:, 8 * t0 : 8 * (t0 + tw)].rearrange("p (vt t) -> p vt t", vt=8),
            )
            return xt_t

        xt_tiles = {0: xchunk_dma(0)}
        basis_sb = const.tile([128, 8, 128], f16)
        nc.sync.dma_start(basis_sb[:], basis_d.rearrange("p (vt n) -> p vt n", vt=8))
        xt_tiles[1] = xchunk_dma(1)
        gp_sb = const.tile([128, 2, 128], f16)
        nc.sync.dma_start(gp_sb[:], gp_d.rearrange("p (ct n) -> p ct n", ct=2))
        mask4_sb = const.tile([128, 2 * N_DIAG * 128], f16)
        nc.sync.dma_start(mask4_sb[:, : N_DIAG * 128], mask2_d[:])
        nc.sync.dma_start(mask4_sb[:, N_DIAG * 128 :], mask2_d[:])
        basisT_sb = const.tile([128, V], f16)
        nc.sync.dma_start(basisT_sb[:], basisT_d[:])
        xt_tiles[2] = xchunk_dma(2)
        xt_tiles[3] = xchunk_dma(3)
        xt_tiles[4] = xchunk_dma(4)
        xt_tiles[5] = xchunk_dma(5)

        # ---- persistent activations ----
        xb_sb = persist.tile([128, T_LOC], f16)              # basis-space x^T
        gq_sb = persist.tile([128, T_OUT], f16)              # G'^T xb
        vo_sb = persist.tile([128, N_BLK, 128], f16)         # xb^T P (t-major)

        # PE warmup / keep-alive: dummy matmuls on a memset scratch tile.
        wu_sb = const.tile([128, 256], f16)
        nc.gpsimd.memset(wu_sb[:], 0.0)

        def keep_alive(n):
            for _ in range(n):
                ka_ps = ps.tile([128, 512], f32, tag="mm")
                nc.tensor.matmul(
                    ka_ps[:, 0:256], wu_sb[:, 0:128],
                    wu_sb[:, 0:256], start=True, stop=True,
                )

        def project_xb(tci):
            t0 = sum(T_CHUNKS[:tci])
            tw = T_CHUNKS[tci]
            xt_t = xt_tiles.pop(tci)
            xb_ps = ps.tile([128, tw], f32, tag="mm")
            for vt in range(8):
                nc.tensor.matmul(
                    xb_ps[:], basis_sb[:, vt, :], xt_t[:, vt, :],
                    start=(vt == 0), stop=(vt == 7),
                )
            # first two chunk evacs on DVE: ACT is busy with its one-time
            # activation-table load at kernel start.
            if tci < 2:
                nc.vector.tensor_copy(xb_sb[:, t0 : t0 + tw], xb_ps[:])
            else:
                nc.scalar.copy(xb_sb[:, t0 : t0 + tw], xb_ps[:])

        def project_gq(tci):
            t0 = sum(T_CHUNKS[:tci])
            tw = min(T_CHUNKS[tci], T_OUT - t0)
            if tw <= 0:
                return
            gq_ps = ps.tile([128, tw], f32, tag="mm")
            nc.tensor.matmul(
                gq_ps[:], gp_sb[:, 0, :], xb_sb[:, t0 : t0 + tw],
                start=True, stop=True,
            )
            if tci < 2:
                nc.vector.tensor_copy(gq_sb[:, t0 : t0 + tw], gq_ps[:])
            else:
                nc.scalar.copy(gq_sb[:, t0 : t0 + tw], gq_ps[:])

        def project_vo(b0, nb):
            # nb vo blocks ([128,128] each) batched into one PSUM bank and
            # evacuated with a single wide ACT copy.
            vo_ps = ps.tile([128, nb * 128], f32, tag="mm")
            for i in range(nb):
                a = (b0 + i) * 128
                nc.tensor.matmul(
                    vo_ps[:, i * 128 : (i + 1) * 128],
                    xb_sb[:, a : a + 128], gp_sb[:, 1, :],
                    start=(i == 0), stop=(i == nb - 1),
                )
            nc.scalar.copy(
                vo_sb[:, b0 : b0 + nb, :].rearrange("p b n -> p (b n)"), vo_ps[:]
            )

        # ---- attention, two query-chunks per stage ----
        sT_q = {}
        rb_q = {}

        def stage_s(pi):
            q0 = pi * 256
            s_ps = pss.tile([128, 4 * 128], f32, tag="s")
            first = True
            for half in range(2):
                for d in range(N_DIAG):
                    s0 = q0 + half * 128 + d * 128
                    nc.tensor.matmul(
                        s_ps[:, (half * 2 + d) * 128 : (half * 2 + d + 1) * 128],
                        xb_sb[:, s0 : s0 + 128],
                        gq_sb[:, q0 + half * 128 : q0 + (half + 1) * 128],
                        start=first, stop=(half == 1 and d == N_DIAG - 1),
                    )
                    first = False
            sT_sb = sT_pool.tile([128, 4 * 128], f16, tag="sT")
            nc.vector.tensor_mul(sT_sb[:], s_ps[:], mask4_sb[:])
            sT_q[pi] = sT_sb

        def stage_pv(pi):
            q0 = pi * 256
            sT_sb = sT_q.pop(pi)
            rb_ps4 = pss.tile([128, 512], f32, tag="s")
            rb_ps = rb_ps4[:, 0:256]
            first = True
            for half in range(2):
                for d in range(N_DIAG):
                    nc.tensor.matmul(
                        rb_ps[:, half * 128 : (half + 1) * 128],
                        vo_sb[:, q0 // 128 + half + d, :],
                        sT_sb[:, (half * 2 + d) * 128 : (half * 2 + d + 1) * 128],
                        start=first, stop=(half == 1 and d == N_DIAG - 1),
                    )
                    first = False
            rb_sb = rb_pool.tile([128, 256], f16)
            nc.scalar.copy(rb_sb[:], rb_ps[:])
            rb_q[pi] = rb_sb

        def stage_y(pi):
            q0 = pi * 256
            rb_sb = rb_q.pop(pi)
            for half in range(2):
                y_ps = py.tile([128, 1024], f32, tag="y")
                for vh in range(2):
                    nc.tensor.matmul(
                        y_ps[:, vh * 512 : (vh + 1) * 512],
                        rb_sb[:, half * 128 : (half + 1) * 128],
                        basisT_sb[:, vh * 512 : (vh + 1) * 512],
                        start=True, stop=True, skip_group_check=True,
                    )
                y_sb = y_pool.tile([128, V], f16)
                if half == 0:
                    nc.vector.tensor_copy(y_sb[:], y_ps[:])
                else:
                    nc.scalar.copy(y_sb[:], y_ps[:])
                nc.sync.dma_start(
                    out_d[q0 + half * 128 : q0 + (half + 1) * 128, :], y_sb[:]
                )

        def band(pairs):
            for pi in pairs:
                stage_s(pi)
                stage_pv(pi)
                stage_y(pi)

        # ---- emission (priority) order ----
        keep_alive(N_WARM)
        project_xb(0)
        project_xb(1)
        project_gq(0)
        project_gq(1)
        keep_alive(N_KA)          # bridge the chunk-2 stream stall
        project_xb(2)
        project_gq(2)
        project_vo(0, 4)
        stage_s(0)
        project_xb(3)
        stage_pv(0)
        project_gq(3)
        project_vo(4, 4)
        stage_s(1)
        stage_y(0)
        stage_pv(1)
        stage_s(2)
        project_xb(4)
        stage_y(1)
        stage_pv(2)
        project_gq(4)
        project_vo(8, 4)
        stage_s(3)
        stage_y(2)
        stage_pv(3)
        stage_s(4)
        project_xb(5)
        stage_y(3)
        stage_pv(4)
        project_gq(5)
        project_vo(12, 4)
        project_vo(16, 1)
        stage_s(5)
        stage_y(4)
        stage_pv(5)
        stage_s(6)
        stage_y(5)
        stage_pv(6)
        stage_s(7)
        stage_y(6)
        stage_pv(7)
        stage_y(7)

    nc.compile()
    return nc


_NC_CACHE = None


def _get_nc():
    global _NC_CACHE
    if _NC_CACHE is None:
        _NC_CACHE = _build_nc()
    return _NC_CACHE


def kernel(x, basis, q_coeffs, k_coeffs, v_coeffs, o_coeffs, decay_logit, out_scale):
    from concourse.bass_utils import run_bass_kernel_spmd

    x = np.asarray(x, dtype=np.float32)
    basis = np.ascontiguousarray(np.asarray(basis, dtype=np.float32))
    decay = float(1.0 / (1.0 + np.exp(-np.float64(np.asarray(decay_logit)))))
    oscale = float(np.asarray(out_scale))
    alpha = oscale / Y_SCALE

    p_idx = np.arange(128, dtype=np.float64)
    # full 2D key/query decay mask per diagonal block d:
    #   mask[p, qr] = alpha * decay^(d*128 + p - qr - 1),  d=0 also tri (p>qr)
    blocks = []
    for d in range(N_DIAG):
        e = d * 128.0 + p_idx[:, None] - p_idx[None, :] - 1.0
        blk = alpha * decay ** e
        if d == 0:
            blk = blk * (p_idx[:, None] > p_idx[None, :])
        blocks.append(blk)
    mask2 = np.ascontiguousarray(np.concatenate(blocks, axis=1).astype(np.float16))

    def pack_rows(a):
        # [(nt*128), m] -> [128, nt*m]  (partition-major, tile index on free)
        nt = a.shape[0] // 128
        return np.ascontiguousarray(
            a.reshape(nt, 128, a.shape[1]).transpose(1, 0, 2).reshape(128, -1)
        ).astype(np.float16)

    basisT = np.ascontiguousarray(basis.T).astype(np.float16)
    basisp = pack_rows(basis)
    # G'[n',n] = sum_c qco[c,n'] kco[c,n];  P[n,m] = sum_c vco[c,n] oco[c,m]
    qc = np.asarray(q_coeffs, dtype=np.float32)
    kc = np.asarray(k_coeffs, dtype=np.float32)
    vc = np.asarray(v_coeffs, dtype=np.float32)
    oc = np.asarray(o_coeffs, dtype=np.float32)
    gmat = (qc.T @ kc).astype(np.float16)     # [128, 128]
    pmat = (vc.T @ oc).astype(np.float16)     # [128, 128]
    gpp = np.ascontiguousarray(np.concatenate([gmat, pmat], axis=1))

    in_maps = []
    for core in range(N_CORES):
        b, h = core // 2, core % 2
        lo = h * T_OUT
        hi = min(T, lo + T_LOC)
        xs = np.zeros((T_LOC, V), dtype=np.float32)
        xs[: hi - lo] = x[b, lo:hi]
        # pack x^T into per-chunk-contiguous SBUF layout:
        # xtp[p, 8*t0 + vt*tw + t] = x[t0+t, vt*128+p] for chunk (t0, tw)
        xtt = xs.T.reshape(8, 128, T_LOC).transpose(1, 0, 2)  # [128, vt, t]
        pieces = []
        t0 = 0
        for tw in T_CHUNKS:
            pieces.append(xtt[:, :, t0 : t0 + tw].reshape(128, 8 * tw))
            t0 += tw
        xtp = np.ascontiguousarray(np.concatenate(pieces, axis=1)).astype(np.float16)
        in_maps.append(
            {
                "xtp": xtp,
                "basisp": basisp,
                "basisT": basisT,
                "gpp": gpp,
                "mask2": mask2,
            }
        )

    nc = _get_nc()
    trace = bool(int(os.environ.get("KERNEL_TRACE", "0")))
    res = run_bass_kernel_spmd(nc, in_maps, list(range(N_CORES)), trace=trace)
    LAST["exec_time_ns"] = res.exec_time_ns
    LAST["results"] = res

    out = np.empty((B, T, V), dtype=np.float32)
    for core in range(N_CORES):
        b, h = core // 2, core % 2
        out[b, h * T_OUT : (h + 1) * T_OUT] = (
            res.results[core]["out"].astype(np.float32) * Y_SCALE
        )
    return out
